# revision 3
# baseline (speedup 1.0000x reference)
"""Trainium2 Bass kernel for nn_Block (dense transformer block, sigmoid attention).

Sharding: 8 cores = 2 (batch) x 4 (query-chunk of 512 tokens).
Host rotates the token axis per core so each core's query chunk is tokens
[0, 512) of its rotated view; K/V are computed over all 2048 (rotated) tokens.
Attention output is invariant to key-token order, so rotation is safe as long
as the coulomb matrix columns are rotated identically.

On-chip layout is feature-major ("F layout"): activations live as x^T with
features on SBUF partitions and tokens on the free axis, so every matmul
contracts along partitions with the weight stationary.

LayerNorm gains/biases are folded into the downstream weights on the host:
    h = z * g + b  (z = (x - mean) * rstd)
    h @ W + bw  ==  z @ (diag(g) W)  +  (b @ W + bw)
so the kernel only ever computes z.
"""
import numpy as np
import ml_dtypes
from contextlib import ExitStack

import concourse.bacc as bacc
import concourse.mybir as mybir
import concourse.tile as tile
from concourse.bass_utils import run_bass_kernel_spmd

F32 = mybir.dt.float32
F32R = mybir.dt.float32r
BF16 = mybir.dt.bfloat16
AF = mybir.ActivationFunctionType
ALU = mybir.AluOpType

B, T, C, H, D = 2, 2048, 512, 8, 64
TQ = 512          # query tokens per core
P = 128
KC = C // P       # 4   C partition-chunks
NT = T // 512     # 4   T tiles of 512
NTK = T // P      # 16  key-token chunks of 128
C4 = 4 * C        # 2048
KC4 = C4 // P     # 16
EPS = 1e-5
N_CORES = 8

_BUILT = None


def _build():
    nc = bacc.Bacc("TRN2", target_bir_lowering=False, debug=False)

    xT_d = nc.dram_tensor("xT", [P, KC, T], F32R, kind="ExternalInput")
    coulT_d = nc.dram_tensor("coulT", [NTK, P, TQ], BF16, kind="ExternalInput")
    wq_d = nc.dram_tensor("wq", [P, KC, C], F32R, kind="ExternalInput")
    wk_d = nc.dram_tensor("wk", [P, KC, C], F32R, kind="ExternalInput")
    wv_d = nc.dram_tensor("wv", [P, KC, C], F32R, kind="ExternalInput")
    wself_d = nc.dram_tensor("wself", [P, KC, C], F32R, kind="ExternalInput")
    wproj_d = nc.dram_tensor("wproj", [P, KC, C], F32R, kind="ExternalInput")
    wfc_d = nc.dram_tensor("wfc", [P, KC, C4], F32R, kind="ExternalInput")
    wfcp_d = nc.dram_tensor("wfcp", [P, KC4, C], BF16, kind="ExternalInput")
    bq_d = nc.dram_tensor("bq", [P, KC], F32, kind="ExternalInput")
    bk_d = nc.dram_tensor("bk", [P, KC], F32, kind="ExternalInput")
    bv_d = nc.dram_tensor("bv", [1, C], F32R, kind="ExternalInput")
    bself_d = nc.dram_tensor("bself", [P, KC], F32, kind="ExternalInput")
    bproj_d = nc.dram_tensor("bproj", [P, KC], F32, kind="ExternalInput")
    bfc_d = nc.dram_tensor("bfc", [P, KC4], F32, kind="ExternalInput")
    bfcp_d = nc.dram_tensor("bfcp", [P, KC], F32, kind="ExternalInput")
    cst_d = nc.dram_tensor("cst", [P, 2], F32R, kind="ExternalInput")  # [ones, 1/C]
    onesr_d = nc.dram_tensor("onesr", [1, P], F32R, kind="ExternalInput")
    outT_d = nc.dram_tensor("outT", [P, KC, TQ], F32, kind="ExternalOutput")

    with tile.TileContext(nc) as tc, ExitStack() as cst_ctx:
        cst = cst_ctx.enter_context(tc.tile_pool(name="cst", bufs=1))
        lateP = cst_ctx.enter_context(tc.tile_pool(name="lateP", bufs=1))
        w12P = cst_ctx.enter_context(tc.tile_pool(name="w12P", bufs=1))
        zP = cst_ctx.enter_context(tc.tile_pool(name="zP", bufs=1))

        # ---- constants / biases ----
        cst_sb = cst.tile([P, 2], F32R)
        nc.sync.dma_start(cst_sb, cst_d[:, :])
        ones_col = cst_sb[:, 0:1]
        cm_col = cst_sb[:, 1:2]
        onesr_sb = cst.tile([1, P], F32R)
        nc.sync.dma_start(onesr_sb, onesr_d[:, :])
        eps1 = cst.tile([1, 1], F32)
        nc.vector.memset(eps1, EPS)
        bq_sb = cst.tile([P, KC], F32)
        bk_sb = cst.tile([P, KC], F32)
        bself_sb = cst.tile([P, KC], F32)
        bproj_sb = cst.tile([P, KC], F32)
        bfc_sb = cst.tile([P, KC4], F32)
        bfcp_sb = cst.tile([P, KC], F32)
        bv_sb = cst.tile([1, C], F32R)
        for sb, d in ((bq_sb, bq_d), (bk_sb, bk_d), (bself_sb, bself_d),
                      (bproj_sb, bproj_d), (bfc_sb, bfc_d), (bfcp_sb, bfcp_d)):
            nc.sync.dma_start(sb, d[:, :])
        nc.sync.dma_start(bv_sb, bv_d[:, :])

        # ---- attention-side weights (close after proj) ----
        wq_sb = w12P.tile([P, KC, C], F32R)
        wk_sb = w12P.tile([P, KC, C], F32R)
        wv_sb = w12P.tile([P, KC, C], F32R)
        wself_sb = w12P.tile([P, KC, C], F32R)
        wproj_sb = w12P.tile([P, KC, C], F32R)
        for sb, d in ((wq_sb, wq_d), (wk_sb, wk_d), (wv_sb, wv_d),
                      (wself_sb, wself_d), (wproj_sb, wproj_d)):
            for kc in range(KC):
                nc.sync.dma_start(sb[:, kc], d[:, kc])

        z_sb = zP.tile([P, KC, T], F32R)

        # =========== Phase 1: load x, LayerNorm 1 (over all T tokens) ========
        with tc.tile_pool(name="ln1P", bufs=1) as ln1P, \
             tc.tile_pool(name="ln1W", bufs=3) as ln1W, \
             tc.tile_pool(name="psLN", bufs=2, space="PSUM") as psLN:
            x_sb = ln1P.tile([P, KC, T], F32R)
            for kc in range(KC):
                nc.sync.dma_start(x_sb[:, kc], xT_d[:, kc])

            # mean row via (1/C)-scaled ones matmul
            m_row = ln1P.tile([1, T], F32)
            for n in range(NT):
                ps_m = psLN.tile([1, 512], F32, tag="st")
                for kc in range(KC):
                    nc.tensor.matmul(ps_m, lhsT=cm_col,
                                     rhs=x_sb[:, kc, n * 512:(n + 1) * 512],
                                     start=(kc == 0), stop=(kc == KC - 1))
                nc.vector.tensor_copy(m_row[:, n * 512:(n + 1) * 512], ps_m)
            m_b = ln1P.tile([P, T], F32)
            nc.gpsimd.partition_broadcast(m_b, m_row)

            # z1 = x - mean  (write f32r)
            for kc in range(KC):
                nc.vector.tensor_tensor(out=z_sb[:, kc], in0=x_sb[:, kc].bitcast(F32),
                                        in1=m_b, op=ALU.subtract)

            # centered variance via squares
            var_row = ln1P.tile([1, T], F32)
            for n in range(NT):
                ps_v = psLN.tile([1, 512], F32, tag="st")
                for kc in range(KC):
                    sq_t = ln1W.tile([P, 512], F32R, tag="sq")
                    nc.scalar.square(sq_t, z_sb[:, kc, n * 512:(n + 1) * 512].bitcast(F32))
                    nc.tensor.matmul(ps_v, lhsT=cm_col, rhs=sq_t,
                                     start=(kc == 0), stop=(kc == KC - 1))
                nc.vector.tensor_copy(var_row[:, n * 512:(n + 1) * 512], ps_v)
            # rs = exp(-0.5 * ln(var + eps))
            nc.scalar.activation(var_row, var_row, AF.Ln, bias=eps1)
            nc.scalar.activation(var_row, var_row, AF.Exp, scale=-0.5)
            rs_b = ln1P.tile([P, T], F32)
            nc.gpsimd.partition_broadcast(rs_b, var_row)
            for kc in range(KC):
                nc.vector.tensor_tensor(out=z_sb[:, kc], in0=z_sb[:, kc].bitcast(F32),
                                        in1=rs_b, op=ALU.mult)

        # =========== Phase 2: q/k/v projections =============================
        with ExitStack() as qctx:
            qkvP = qctx.enter_context(tc.tile_pool(name="qkvP", bufs=1))
            q_sb = qkvP.tile([P, KC, TQ], BF16)
            k_sb = qkvP.tile([P, KC, T], BF16)
            v_sb = qkvP.tile([P, NTK, C], BF16)
            with tc.tile_pool(name="psMM", bufs=4, space="PSUM") as psMM:
                for mo in range(KC):
                    ps = psMM.tile([P, 512], F32, tag="mm")
                    for kc in range(KC):
                        nc.tensor.matmul(ps, lhsT=wq_sb[:, kc, mo * P:(mo + 1) * P],
                                         rhs=z_sb[:, kc, 0:TQ],
                                         start=(kc == 0), stop=(kc == KC - 1))
                    nc.vector.tensor_scalar(q_sb[:, mo], ps, bq_sb[:, mo:mo + 1],
                                            None, ALU.add)
                for mo in range(KC):
                    for n in range(NT):
                        ps = psMM.tile([P, 512], F32, tag="mm")
                        for kc in range(KC):
                            nc.tensor.matmul(ps, lhsT=wk_sb[:, kc, mo * P:(mo + 1) * P],
                                             rhs=z_sb[:, kc, n * 512:(n + 1) * 512],
                                             start=(kc == 0), stop=(kc == KC - 1))
                        nc.vector.tensor_scalar(k_sb[:, mo, n * 512:(n + 1) * 512], ps,
                                                bk_sb[:, mo:mo + 1], None, ALU.add)
                for ts_ in range(NTK):
                    ps = psMM.tile([P, 512], F32, tag="mm")
                    for kc in range(KC):
                        nc.tensor.matmul(ps, lhsT=z_sb[:, kc, ts_ * P:(ts_ + 1) * P],
                                         rhs=wv_sb[:, kc],
                                         start=(kc == 0), stop=False)
                    nc.tensor.matmul(ps, lhsT=onesr_sb, rhs=bv_sb,
                                     start=False, stop=True)
                    nc.scalar.activation(v_sb[:, ts_], ps, AF.Copy)

            # =========== Phase 3: attention =================================
            with tc.tile_pool(name="attW", bufs=1) as attW, \
                 tc.tile_pool(name="attS", bufs=2) as attS, \
                 tc.tile_pool(name="attC", bufs=3) as attC, \
                 tc.tile_pool(name="psATT", bufs=1, space="PSUM") as psATT:
                y_ps = [psATT.tile([P, TQ], F32, tag=f"y{j}", name=f"y_ps{j}")
                        for j in range(KC)]
                for tkc in range(NTK):
                    coul_t = attC.tile([P, TQ], BF16, tag="coul")
                    nc.sync.dma_start(coul_t, coulT_d[tkc])
                    s_t = attS.tile([P, H, TQ], BF16, tag="st")
                    for half in range(2):
                        sc_ps = psATT.tile([P, 4, TQ], F32, tag="sc")
                        for hh in range(4):
                            h = half * 4 + hh
                            chk, po = h // 2, 64 * (h % 2)
                            nc.tensor.matmul(
                                sc_ps[:, hh, :],
                                lhsT=k_sb[po:po + 64, chk, tkc * P:(tkc + 1) * P],
                                rhs=q_sb[po:po + 64, chk, :],
                                start=True, stop=True)
                        nc.scalar.activation(s_t[:, half * 4:half * 4 + 4, :], sc_ps,
                                             AF.Sigmoid, scale=0.125)
                    nc.vector.tensor_tensor(
                        out=s_t, in0=s_t,
                        in1=coul_t[:, None, :].to_broadcast([P, H, TQ]),
                        op=ALU.mult)
                    for h in range(H):
                        j, po = h // 2, 64 * (h % 2)
                        nc.tensor.matmul(
                            y_ps[j][po:po + 64, :],
                            lhsT=v_sb[:, tkc, 64 * h:64 * h + 64],
                            rhs=s_t[:, h, :],
                            start=(tkc == 0), stop=False,
                            tile_position=(0, po))

                # ===== Phase 4: self-path accumulate + y2 ====================
                y2_sb = lateP.tile([P, KC, TQ], F32R, tag="mid_a")
                for j in range(KC):
                    for kc in range(KC):
                        nc.tensor.matmul(y_ps[j],
                                         lhsT=wself_sb[:, kc, j * P:(j + 1) * P],
                                         rhs=z_sb[:, kc, 0:TQ],
                                         start=False, stop=(kc == KC - 1))
                    nc.vector.tensor_scalar(y2_sb[:, j], y_ps[j],
                                            bself_sb[:, j:j + 1], None, ALU.add)

        # =========== Phase 5: out-proj ======================================
        y3_sb = lateP.tile([P, KC, TQ], F32R, tag="mid_b")
        with tc.tile_pool(name="psP5", bufs=2, space="PSUM") as psP5:
            for j in range(KC):
                ps = psP5.tile([P, 512], F32, tag="mm")
                for kc in range(KC):
                    nc.tensor.matmul(ps, lhsT=wproj_sb[:, kc, j * P:(j + 1) * P],
                                     rhs=y2_sb[:, kc],
                                     start=(kc == 0), stop=(kc == KC - 1))
                nc.vector.tensor_scalar(y3_sb[:, j], ps, bproj_sb[:, j:j + 1],
                                        None, ALU.add)

        # =========== Phase 6: LayerNorm 2 (over TQ tokens) ==================
        z2_sb = lateP.tile([P, KC, TQ], F32R, tag="mid_c")
        with tc.tile_pool(name="ln2P", bufs=1) as ln2P, \
             tc.tile_pool(name="ln2W", bufs=2) as ln2W, \
             tc.tile_pool(name="psLN2", bufs=2, space="PSUM") as psLN2:
            m2_row = ln2P.tile([1, TQ], F32)
            ps_m2 = psLN2.tile([1, 512], F32, tag="st2")
            for kc in range(KC):
                nc.tensor.matmul(ps_m2, lhsT=cm_col, rhs=y3_sb[:, kc],
                                 start=(kc == 0), stop=(kc == KC - 1))
            nc.vector.tensor_copy(m2_row, ps_m2)
            m2_b = ln2P.tile([P, TQ], F32)
            nc.gpsimd.partition_broadcast(m2_b, m2_row)
            for kc in range(KC):
                nc.vector.tensor_tensor(out=z2_sb[:, kc], in0=y3_sb[:, kc].bitcast(F32),
                                        in1=m2_b, op=ALU.subtract)
            var2_row = ln2P.tile([1, TQ], F32)
            ps_v2 = psLN2.tile([1, 512], F32, tag="st2")
            for kc in range(KC):
                sq_t = ln2W.tile([P, 512], F32R, tag="sq2")
                nc.scalar.square(sq_t, z2_sb[:, kc].bitcast(F32))
                nc.tensor.matmul(ps_v2, lhsT=cm_col, rhs=sq_t,
                                 start=(kc == 0), stop=(kc == KC - 1))
            nc.vector.tensor_copy(var2_row, ps_v2)
            nc.scalar.activation(var2_row, var2_row, AF.Ln, bias=eps1)
            nc.scalar.activation(var2_row, var2_row, AF.Exp, scale=-0.5)
            rs2_b = ln2P.tile([P, TQ], F32)
            nc.gpsimd.partition_broadcast(rs2_b, var2_row)
            for kc in range(KC):
                nc.vector.tensor_tensor(out=z2_sb[:, kc], in0=z2_sb[:, kc].bitcast(F32),
                                        in1=rs2_b, op=ALU.mult)

        # =========== Phase 7/8: MLP =========================================
        with tc.tile_pool(name="wts3", bufs=1) as wts3, \
             tc.tile_pool(name="gP", bufs=1) as gP, \
             tc.tile_pool(name="psMLP", bufs=4, space="PSUM") as psMLP:
            wfc_sb = wts3.tile([P, KC, C4], F32R)
            for kc in range(KC):
                nc.sync.dma_start(wfc_sb[:, kc], wfc_d[:, kc])
            wfcp_sb = wts3.tile([P, KC4, C], BF16)
            for kc in range(0, KC4, 4):
                nc.sync.dma_start(wfcp_sb[:, kc:kc + 4], wfcp_d[:, kc:kc + 4])

            g_sb = gP.tile([P, KC4, TQ], BF16)
            for mo in range(KC4):
                ps = psMLP.tile([P, 512], F32, tag="mm")
                for kc in range(KC):
                    nc.tensor.matmul(ps, lhsT=wfc_sb[:, kc, mo * P:(mo + 1) * P],
                                     rhs=z2_sb[:, kc],
                                     start=(kc == 0), stop=(kc == KC - 1))
                nc.scalar.activation(g_sb[:, mo], ps, AF.Gelu,
                                     bias=bfc_sb[:, mo:mo + 1])
            out_sb = gP.tile([P, KC, TQ], F32)
            for j in range(KC):
                ps = psMLP.tile([P, 512], F32, tag="mm")
                for kc in range(KC4):
                    nc.tensor.matmul(ps, lhsT=wfcp_sb[:, kc, j * P:(j + 1) * P],
                                     rhs=g_sb[:, kc],
                                     start=(kc == 0), stop=(kc == KC4 - 1))
                nc.vector.tensor_scalar(out_sb[:, j], ps, bfcp_sb[:, j:j + 1],
                                        None, ALU.add)
            nc.sync.dma_start(outT_d[:, :, :], out_sb)

    nc.compile()
    return nc


def _get_nc():
    global _BUILT
    if _BUILT is None:
        _BUILT = _build()
    return _BUILT


def _fmt_lhs(w):
    """[Cin, Cout] -> [128, Cin//128, Cout] partition-major lhsT layout."""
    return np.ascontiguousarray(
        w.reshape(w.shape[0] // P, P, w.shape[1]).transpose(1, 0, 2))


def _fmt_bias(b):
    """[O] -> [128, O//128] per-partition layout."""
    return np.ascontiguousarray(b.reshape(-1, P).T)


def _prep(inputs):
    f32 = np.float32
    x = np.asarray(inputs["x"], f32)
    coul = np.asarray(inputs["coulomb_matrix"], f32)
    g1 = np.asarray(inputs["ln1_g"], f32)
    b1 = np.asarray(inputs["ln1_b"], f32)
    g2 = np.asarray(inputs["ln2_g"], f32)
    b2 = np.asarray(inputs["ln2_b"], f32)
    wattn = np.asarray(inputs["w_attn"], f32)
    battn = np.asarray(inputs["b_attn"], f32)
    w_self = np.asarray(inputs["w_self"], f32)
    b_self = np.asarray(inputs["b_self"], f32)
    w_proj = np.asarray(inputs["w_proj"], f32)
    b_proj = np.asarray(inputs["b_proj"], f32)
    w_fc = np.asarray(inputs["w_fc"], f32)
    b_fc = np.asarray(inputs["b_fc"], f32)
    w_fcp = np.asarray(inputs["w_fc_proj"], f32)
    b_fcp = np.asarray(inputs["b_fc_proj"], f32)

    wq, wk, wv = wattn[:, 0:C], wattn[:, C:2 * C], wattn[:, 2 * C:]
    shared = {
        "wq": _fmt_lhs(g1[:, None] * wq),
        "wk": _fmt_lhs(g1[:, None] * wk),
        "wv": _fmt_lhs(g1[:, None] * wv),
        "wself": _fmt_lhs(g1[:, None] * w_self),
        "wproj": _fmt_lhs(w_proj),
        "wfc": _fmt_lhs(g2[:, None] * w_fc),
        "wfcp": _fmt_lhs(w_fcp).astype(ml_dtypes.bfloat16),
        "bq": _fmt_bias(battn[0:C] + b1 @ wq),
        "bk": _fmt_bias(battn[C:2 * C] + b1 @ wk),
        "bv": (battn[2 * C:] + b1 @ wv).reshape(1, C),
        "bself": _fmt_bias(b_self + b1 @ w_self),
        "bproj": _fmt_bias(b_proj),
        "bfc": _fmt_bias(b_fc + b2 @ w_fc),
        "bfcp": _fmt_bias(b_fcp),
        "cst": np.stack([np.ones(P, f32), np.full(P, 1.0 / C, f32)], axis=1),
        "onesr": np.ones((1, P), f32),
    }
    in_maps = []
    for core in range(N_CORES):
        b, tqi = divmod(core, 4)
        tq0 = tqi * TQ
        xr = np.roll(x[b], -tq0, axis=0)                      # [T, C]
        xT = np.ascontiguousarray(
            xr.T.reshape(KC, P, T).transpose(1, 0, 2))        # [P, KC, T]
        cr = np.roll(coul[b], -tq0, axis=1)[tq0:tq0 + TQ, :]  # [TQ, T]
        coulT = np.ascontiguousarray(
            cr.T.reshape(NTK, P, TQ)).astype(ml_dtypes.bfloat16)
        m = dict(shared)
        m["xT"] = xT
        m["coulT"] = coulT
        in_maps.append(m)
    return in_maps


def _assemble(results):
    out = np.empty((B, T, C), np.float32)
    for core in range(N_CORES):
        b, tqi = divmod(core, 4)
        tq0 = tqi * TQ
        r = results[core]["outT"]                  # [P, KC, TQ]
        o = r.transpose(1, 0, 2).reshape(C, TQ).T  # [TQ, C]
        out[b, tq0:tq0 + TQ] = o
    return out


def _run(inputs, trace=False):
    nc = _get_nc()
    in_maps = _prep(inputs)
    res = run_bass_kernel_spmd(nc, in_maps, core_ids=list(range(N_CORES)),
                               trace=trace)
    return _assemble(res.results), res


def kernel(**inputs):
    out, _ = _run(inputs)
    return out
